# revision 1
# baseline (speedup 1.0000x reference)
"""CFConv (SchNet continuous-filter convolution) — Bass/Tile kernel for
8 Trainium2 NeuronCores.

Contract: kernel(**inputs) takes the FULL unsharded inputs (as produced by
reference.setup_inputs()) and returns the FULL [16, 256, 128] f32 output.

Sharding: data-parallel over the batch dim (B=16 -> 2 batches per core).
Host-side work is limited to layout/sharding transforms (slicing per core,
an axis transpose of f_ij, int index reformatting); all arithmetic runs on
the NeuronCores.

Per-core kernel (F-on-partitions layout; rows r = atom*64 + neighbor):
  y table:  psY[f, a] = w_in2f.T @ x_t                      (PE)
  mm1:      psH[h, r] = fw1.T @ f_ij_t                      (PE)
  ssp:      H = Ln(0.5*Exp(psH + fb1) + 0.5)                (ACT, one table)
  mm2:      psW[f, r] = fw2.T @ H                           (PE)
  gather:   ynbh[f, r] = ytab[f, neighbors(r)]              (GPSIMD ap_gather)
  s:        cutoff(r_ij)*mask  -> bf16 row -> partition_broadcast
  product:  P = ((psW + fb2) * ynbh) * s_bcast              (DVE STT + TT)
  reduce+out: psO[o, a] = sum_n w_f2out.T @ P[:, n::64]     (64 accum MMs)
  out = ssp(psO + b_f2out) -> PE-transpose -> [a, o]
"""
import math
import numpy as np
import concourse.bass as bass
import concourse.bacc as bacc
import concourse.mybir as mybir
from concourse import tile
from concourse import bass2jax

f32 = mybir.dt.float32
bf16 = mybir.dt.bfloat16
i16 = mybir.dt.int16
i32 = mybir.dt.int32
AF = mybir.ActivationFunctionType
ALU = mybir.AluOpType

B, A, N, G, F = 16, 256, 64, 64, 128
R = A * N
CHUNK = 2048
NCH = R // CHUNK
CUTOFF = 5.0
N_CORES = 8
BPC = B // N_CORES


def _host_prep(inputs, n_cores=N_CORES):
    bpc = BPC
    f_ij = np.asarray(inputs["f_ij"], np.float32)
    fij_t = np.ascontiguousarray(f_ij.reshape(B, R, G).transpose(0, 2, 1))
    nbr_flat = np.asarray(inputs["neighbors"]).reshape(B, R).astype(np.int16)
    wrapped = nbr_flat.reshape(B, R // 16, 16).transpose(0, 2, 1)
    idx = np.ascontiguousarray(np.tile(wrapped, (1, 8, 1)).astype(np.int16))
    x = np.ascontiguousarray(np.asarray(inputs["x"], np.float32))
    r_ij = np.ascontiguousarray(np.asarray(inputs["r_ij"], np.float32))
    mask = np.ascontiguousarray(np.asarray(inputs["pairwise_mask"], np.float32))
    ident = np.eye(128, dtype=np.float32)
    sh = lambda t, c: np.ascontiguousarray(t[c * bpc:(c + 1) * bpc])
    w = lambda k: np.ascontiguousarray(np.asarray(inputs[k], np.float32))
    in_maps = []
    for c in range(n_cores):
        in_maps.append({
            "fij_t": sh(fij_t, c), "x": sh(x, c), "r_ij": sh(r_ij, c),
            "mask": sh(mask, c), "idx": sh(idx, c),
            "fw1": w("fw1"), "fb1": w("fb1"), "fw2": w("fw2"), "fb2": w("fb2"),
            "w_in2f": w("w_in2f"), "w_f2out": w("w_f2out"),
            "b_f2out": w("b_f2out"), "ident": ident,
        })
    return in_maps


def build_nc(bpc=BPC, num_devices=N_CORES, reps=1):
    nc = bacc.Bacc("TRN2", target_bir_lowering=False, debug=False,
                   num_devices=num_devices)
    D = nc.declare_dram_parameter
    fij_t_d = D("fij_t", [bpc, G, R], f32, isOutput=False)
    x_d = D("x", [bpc, A, F], f32, isOutput=False)
    r_d = D("r_ij", [bpc, A, N], f32, isOutput=False)
    m_d = D("mask", [bpc, A, N], f32, isOutput=False)
    idx_d = D("idx", [bpc, 128, R // 16], i16, isOutput=False)
    fw1_d = D("fw1", [G, F], f32, isOutput=False)
    fb1_d = D("fb1", [F], f32, isOutput=False)
    fw2_d = D("fw2", [F, F], f32, isOutput=False)
    fb2_d = D("fb2", [F], f32, isOutput=False)
    win_d = D("w_in2f", [F, F], f32, isOutput=False)
    wout_d = D("w_f2out", [F, F], f32, isOutput=False)
    bo_d = D("b_f2out", [F], f32, isOutput=False)
    id_d = D("ident", [128, 128], f32, isOutput=False)
    out_d = D("out", [bpc, A, F], f32, isOutput=True)

    with tile.TileContext(nc) as tc:
        with tc.tile_pool(name="const", bufs=1) as cpool, \
             tc.tile_pool(name="work", bufs=3) as wpool, \
             tc.tile_pool(name="blk", bufs=2) as bpool, \
             tc.tile_pool(name="ps_h", bufs=2, space="PSUM") as ph, \
             tc.tile_pool(name="ps_w", bufs=2, space="PSUM") as pw, \
             tc.tile_pool(name="ps_t", bufs=1, space="PSUM") as pt, \
             tc.tile_pool(name="ps_o", bufs=1, space="PSUM") as po:

            fw1_sb = cpool.tile([G, F], bf16, tag="fw1")
            nc.gpsimd.dma_start(out=fw1_sb[:], in_=fw1_d[:, :])
            fw2_sb = cpool.tile([F, F], bf16, tag="fw2")
            nc.gpsimd.dma_start(out=fw2_sb[:], in_=fw2_d[:, :])
            win_sb = cpool.tile([F, F], bf16, tag="win")
            nc.gpsimd.dma_start(out=win_sb[:], in_=win_d[:, :])
            wout_sb = cpool.tile([F, F], bf16, tag="wout")
            nc.gpsimd.dma_start(out=wout_sb[:], in_=wout_d[:, :])
            id_sb = cpool.tile([128, 128], f32, tag="ident")
            nc.sync.dma_start(out=id_sb[:], in_=id_d[:, :])
            fb1_sb = cpool.tile([F, 1], f32, tag="fb1")
            nc.sync.dma_start(out=fb1_sb[:],
                              in_=fb1_d.rearrange("(p o) -> p o", o=1))
            fb2_sb = cpool.tile([F, 1], f32, tag="fb2")
            nc.sync.dma_start(out=fb2_sb[:],
                              in_=fb2_d.rearrange("(p o) -> p o", o=1))
            bo_sb = cpool.tile([F, 1], f32, tag="bo")
            nc.sync.dma_start(out=bo_sb[:],
                              in_=bo_d.rearrange("(p o) -> p o", o=1))
            half_sb = cpool.tile([128, 1], f32, tag="half")
            nc.gpsimd.memset(half_sb[:], 0.5)
            hpi_sb = cpool.tile([128, 1], f32, tag="hpi")
            nc.gpsimd.memset(hpi_sb[:], math.pi / 2.0)

            for rep in range(reps):
                for b in range(bpc):
                    xt_sb = wpool.tile([128, 256], bf16, tag="xt")
                    for h in range(2):
                        x_sb = wpool.tile([128, 128], f32, tag="xin")
                        nc.sync.dma_start(out=x_sb[:],
                                          in_=x_d[b, 128 * h:128 * (h + 1), :])
                        ps = pt.tile([128, 128], f32, tag="tp")
                        nc.tensor.transpose(ps[:], x_sb[:], id_sb[:])
                        nc.vector.tensor_copy(xt_sb[:, 128 * h:128 * (h + 1)],
                                              ps[:])
                    ytab_sb = wpool.tile([128, 256], f32, tag="ytab")
                    for h in range(2):
                        ps = pt.tile([128, 128], f32, tag="tp")
                        nc.tensor.matmul(ps[:], win_sb[:],
                                         xt_sb[:, 128 * h:128 * (h + 1)],
                                         start=True, stop=True)
                        nc.vector.tensor_copy(ytab_sb[:, 128 * h:128 * (h + 1)],
                                              ps[:])

                    r_sb = wpool.tile([128, 128], f32, tag="rin")
                    nc.sync.dma_start(
                        out=r_sb[:],
                        in_=r_d[b].rearrange("(p q) n -> p (q n)", p=128))
                    m_sb = wpool.tile([128, 128], f32, tag="min")
                    nc.sync.dma_start(
                        out=m_sb[:],
                        in_=m_d[b].rearrange("(p q) n -> p (q n)", p=128))
                    c_sb = wpool.tile([128, 128], f32, tag="c")
                    nc.scalar.activation(c_sb[:], r_sb[:], AF.Sin,
                                         bias=hpi_sb[:],
                                         scale=-math.pi / CUTOFF)
                    nc.vector.tensor_scalar(c_sb[:], c_sb[:], 1.0, 0.5,
                                            ALU.add, ALU.mult)
                    cut_sb = wpool.tile([128, 128], f32, tag="cut")
                    nc.vector.tensor_scalar(cut_sb[:], r_sb[:], CUTOFF, None,
                                            ALU.is_lt)
                    nc.vector.tensor_tensor(c_sb[:], c_sb[:], cut_sb[:],
                                            ALU.mult)
                    s_bf = wpool.tile([128, 128], bf16, tag="sbf")
                    nc.vector.tensor_tensor(s_bf[:], c_sb[:], m_sb[:], ALU.mult)
                    s_row = wpool.tile([1, R], bf16, tag="srow", bufs=1)
                    nc.gpsimd.dma_start(
                        out=s_row.rearrange("o (p q) -> o p q", p=128),
                        in_=s_bf[:])

                    idx_sb = wpool.tile([128, R // 16], i16, tag="idx")
                    nc.sync.dma_start(out=idx_sb[:], in_=idx_d[b])

                    P_sb = bpool.tile([128, R], bf16, tag="P")
                    for c in range(NCH):
                        cs = slice(c * CHUNK, (c + 1) * CHUNK)
                        fij_sb = wpool.tile([G, CHUNK], bf16, tag="fij")
                        nc.gpsimd.dma_start(out=fij_sb[:],
                                            in_=fij_t_d[b, :, cs])
                        H_sb = wpool.tile([128, CHUNK], bf16, tag="H")
                        for j in range(CHUNK // 512):
                            js = slice(j * 512, (j + 1) * 512)
                            psH = ph.tile([128, 512], f32, tag="psH")
                            nc.tensor.matmul(psH[:], fw1_sb[:], fij_sb[:, js],
                                             start=True, stop=True)
                            eH_sb = wpool.tile([128, 512], f32, tag="eH")
                            nc.scalar.activation(eH_sb[:], psH[:], AF.Exp,
                                                 bias=fb1_sb[:], scale=1.0)
                            nc.scalar.activation(H_sb[:, js], eH_sb[:], AF.Ln,
                                                 bias=half_sb[:], scale=0.5)
                        ynbh_sb = wpool.tile([128, CHUNK], f32, tag="ynbh",
                                             bufs=2)
                        nc.gpsimd.ap_gather(
                            out_ap=ynbh_sb.unsqueeze(2),
                            in_ap=ytab_sb.unsqueeze(2),
                            idxs_ap=idx_sb[:, c * (CHUNK // 16):
                                           (c + 1) * (CHUNK // 16)],
                            channels=128, num_elems=A, d=1, num_idxs=CHUNK)
                        sbc_sb = wpool.tile([128, CHUNK], bf16, tag="sbc")
                        nc.gpsimd.partition_broadcast(
                            out_ap=sbc_sb.bitcast(i32),
                            in_ap=s_row[:, cs].bitcast(i32), channels=128)
                        for j in range(CHUNK // 512):
                            js = slice(j * 512, (j + 1) * 512)
                            pjs = slice(c * CHUNK + j * 512,
                                        c * CHUNK + (j + 1) * 512)
                            psW = pw.tile([128, 512], f32, tag="psW")
                            nc.tensor.matmul(psW[:], fw2_sb[:], H_sb[:, js],
                                             start=True, stop=True)
                            t_sb = wpool.tile([128, 512], bf16, tag="t")
                            nc.vector.scalar_tensor_tensor(
                                out=t_sb[:], in0=psW[:], scalar=fb2_sb[:],
                                in1=ynbh_sb[:, js], op0=ALU.add, op1=ALU.mult)
                            nc.vector.tensor_tensor(P_sb[:, pjs], t_sb[:],
                                                    sbc_sb[:, js], ALU.mult)

                    psO = po.tile([128, A], f32, tag="psO")
                    Pv = P_sb.rearrange("p (a n) -> p n a", n=N)
                    for n in range(N):
                        nc.tensor.matmul(psO[:], wout_sb[:], Pv[:, n, :],
                                         start=(n == 0), stop=(n == N - 1))
                    eo_sb = wpool.tile([128, A], f32, tag="eo")
                    nc.scalar.activation(eo_sb[:], psO[:], AF.Exp,
                                         bias=bo_sb[:], scale=1.0)
                    o_sb = wpool.tile([128, A], f32, tag="o")
                    nc.scalar.activation(o_sb[:], eo_sb[:], AF.Ln,
                                         bias=half_sb[:], scale=0.5)
                    for h in range(2):
                        ps = pt.tile([128, 128], f32, tag="tp")
                        nc.tensor.transpose(ps[:],
                                            o_sb[:, 128 * h:128 * (h + 1)],
                                            id_sb[:])
                        oT_sb = wpool.tile([128, 128], f32, tag="oT")
                        nc.vector.tensor_copy(oT_sb[:], ps[:])
                        nc.sync.dma_start(
                            out=out_d[b, 128 * h:128 * (h + 1), :],
                            in_=oT_sb[:])
    nc.compile()
    return nc


_NC_CACHE = {}


def kernel(**inputs) -> np.ndarray:
    in_maps = _host_prep(inputs)
    if "nc" not in _NC_CACHE:
        _NC_CACHE["nc"] = build_nc(bpc=BPC, num_devices=N_CORES, reps=1)
    nc = _NC_CACHE["nc"]
    results = bass2jax.run_bass_via_pjrt(nc, in_maps, n_cores=N_CORES)
    out = np.concatenate([r["out"] for r in results], axis=0)
    return out.astype(np.float32)


if __name__ == "__main__":
    rng = np.random.default_rng(0)
    demo = {
        "x": rng.standard_normal((B, A, F)).astype(np.float32),
        "r_ij": (rng.random((B, A, N)) * 6.0).astype(np.float32),
        "f_ij": rng.random((B, A, N, G)).astype(np.float32),
        "neighbors": rng.integers(0, A, (B, A, N)).astype(np.int64),
        "pairwise_mask": (rng.random((B, A, N)) < 0.9).astype(np.float32),
        "fw1": (rng.standard_normal((G, F)) / math.sqrt(G)).astype(np.float32),
        "fb1": np.zeros(F, np.float32),
        "fw2": (rng.standard_normal((F, F)) / math.sqrt(F)).astype(np.float32),
        "fb2": np.zeros(F, np.float32),
        "w_in2f": (rng.standard_normal((F, F)) / math.sqrt(F)).astype(np.float32),
        "w_f2out": (rng.standard_normal((F, F)) / math.sqrt(F)).astype(np.float32),
        "b_f2out": np.zeros(F, np.float32),
    }
    out = kernel(**demo)
    print("kernel output:", out.shape, out.dtype, float(np.abs(out).max()))

